# revision 1
# baseline (speedup 1.0000x reference)
"""Causal self-attention (B=4, T=1024, C=768, 12 heads) on 8 trn2 cores.

Sharding: core c = (batch b=c//2, head-group hg=c%2 of 6 heads).
Each core: QKV projection for its head-group (TP column split of Wqkv),
causal attention for 6 heads, partial output projection (TP row split of
Wproj). Host sums the two partials per batch (the all-reduce) and
transposes back.

Device-side layouts (contraction dim always on partitions, so no
on-device transposes are needed):
  x^T  [C=768, T=1024]    (prepared host-side)
  q^T/k^T = W^T x^T as [cols, T]  (lhsT=Wqk slice, rhs=x^T)
  v = x W_v as [T, cols]          (lhsT=x^T chunk, rhs=Wv)
  scoresT [T_k, T_q] = k_h q_h^T  (lhsT=k_h^T, rhs=q_h^T)
  softmax without max-subtraction (scores ~ N(0, 0.1); exp is safe),
  denominator via a ones-column appended to v (row 64 of att@[v|1]),
  out_h^T [64, T_q] = [v|1]^T attT (lhsT=v_ext chunk, rhs=attT chunk)
  proj^T [768, T] = Wp_hg^T out^T (lhsT=Wp slice, rhs=out^T)
"""

import numpy as np
import ml_dtypes

B, T, C = 4, 1024, 768
NH, HD = 12, 64
HPC = NH // 2          # heads per core = 6
QKCOLS = 2 * HPC * HD  # 768 (q then k cols for this head group)
VC = HPC * HD          # 384
NCORES = 8
TB = 512               # matmul moving free-dim block
KC = 128               # contraction chunk
BF16 = ml_dtypes.bfloat16

_prog = None  # cached (nc, ) build


def _build_program():
    import concourse.bass as bass
    import concourse.tile as tile
    from concourse import bacc, mybir

    f32 = mybir.dt.float32
    bf16 = mybir.dt.bfloat16

    nc = bacc.Bacc(
        "TRN2", target_bir_lowering=False, debug=False, enable_asserts=False
    )

    xT = nc.dram_tensor("xT", [C, T], bf16, kind="ExternalInput")
    wqk = nc.dram_tensor("wqk", [C, QKCOLS], bf16, kind="ExternalInput")
    wv = nc.dram_tensor("wv", [C, VC], bf16, kind="ExternalInput")
    wp = nc.dram_tensor("wp", [VC, C], bf16, kind="ExternalInput")
    bqk = nc.dram_tensor("bqk", [128, 6], f32, kind="ExternalInput")
    bv = nc.dram_tensor("bv", [128, VC], f32, kind="ExternalInput")
    bp = nc.dram_tensor("bp", [128, 6], f32, kind="ExternalInput")
    masks = nc.dram_tensor("masks", [128, 4 * TB], bf16, kind="ExternalInput")
    out = nc.dram_tensor("out", [C, T], f32, kind="ExternalOutput")

    Exp = mybir.ActivationFunctionType.Exp

    with tile.TileContext(nc) as tc:
        with (
            tc.tile_pool(name="consts", bufs=1) as consts,
            tc.tile_pool(name="psum_mm", bufs=3, space="PSUM") as psum_mm,
            tc.tile_pool(name="psum_acc", bufs=2, space="PSUM") as psum_acc,
            tc.tile_pool(name="psum_bc", bufs=2, space="PSUM") as psum_bc,
            tc.tile_pool(name="work", bufs=1) as work,
        ):
            # ---- load constants / inputs into SBUF ----
            xT_sb = consts.tile([128, 6, T], bf16)
            nc.sync.dma_start(xT_sb[:], xT.rearrange("(a p) t -> p a t", p=128))
            wqk_sb = consts.tile([128, 6, QKCOLS], bf16)
            nc.sync.dma_start(wqk_sb[:], wqk.rearrange("(a p) c -> p a c", p=128))
            wv_sb = consts.tile([128, 6, VC], bf16)
            nc.sync.dma_start(wv_sb[:], wv.rearrange("(a p) c -> p a c", p=128))
            wp_sb = consts.tile([128, 3, C], bf16)
            nc.sync.dma_start(wp_sb[:], wp.rearrange("(a p) c -> p a c", p=128))
            bqk_sb = consts.tile([128, 6], f32)
            nc.sync.dma_start(bqk_sb[:], bqk[:])
            bv_sb = consts.tile([128, VC], f32)
            nc.sync.dma_start(bv_sb[:], bv[:])
            bp_sb = consts.tile([128, 6], f32)
            nc.sync.dma_start(bp_sb[:], bp[:])
            masks_sb = consts.tile([128, 4, TB], bf16)
            nc.sync.dma_start(masks_sb[:], masks.rearrange("p (m f) -> p m f", m=4))

            ones_sb = consts.tile([1, 64], f32)
            nc.vector.memset(ones_sb[:], 1.0)

            qk_sb = consts.tile([128, 6, T], bf16)   # q^T (blocks 0-2), k^T (3-5)
            v_sb = consts.tile([128, 8, HPC, HD + 1], bf16)  # [Tk chunk][head][v|1]
            out_sb = consts.tile([128, 3, T], bf16)  # attention out^T [384, T]

            nc.vector.memset(v_sb[:, :, :, HD : HD + 1], 1.0)

            # ---- phase 1a: q^T / k^T = Wqk^T @ x^T, in [cols, T] layout ----
            for cb in range(6):
                for tb in range(2):
                    ps_qk = psum_mm.tile([128, TB], f32, tag="mm")
                    for kc in range(6):
                        nc.tensor.matmul(
                            ps_qk[:],
                            wqk_sb[:, kc, cb * 128 : (cb + 1) * 128],
                            xT_sb[:, kc, tb * TB : (tb + 1) * TB],
                            start=(kc == 0),
                            stop=(kc == 5),
                        )
                    # + bias (per partition), cast bf16
                    nc.vector.tensor_scalar_add(
                        qk_sb[:, cb, tb * TB : (tb + 1) * TB],
                        ps_qk[:],
                        bqk_sb[:, cb : cb + 1],
                    )

            # ---- phase 1b: v = x @ Wv + bv, in [T, cols] layout ----
            for tk in range(8):
                ps_v = psum_mm.tile([128, VC], f32, tag="mm")
                for kc in range(6):
                    nc.tensor.matmul(
                        ps_v[:],
                        xT_sb[:, kc, tk * 128 : (tk + 1) * 128],
                        wv_sb[:, kc, :],
                        start=(kc == 0),
                        stop=(kc == 5),
                    )
                nc.vector.tensor_add(
                    v_sb[:, tk, :, 0:HD],
                    ps_v.rearrange("p (h d) -> p h d", h=HPC),
                    bv_sb.rearrange("p (h d) -> p h d", h=HPC),
                )

            # ---- phase 2: attention per head ----
            for h in range(HPC):
                hp = (h % 2) * 64          # partition offset within block
                qblk = h // 2              # q block index in qk_sb
                kblk = 3 + h // 2          # k block index in qk_sb
                for qb in range(2):
                    nkb = 4 * (qb + 1)     # causal: T_k chunks needed
                    oe = psum_acc.tile([65, TB], f32, tag="acc")
                    for kb in range(nkb):
                        ps_att = psum_mm.tile([128, TB], f32, tag="mm")
                        nc.tensor.matmul(
                            ps_att[:],
                            qk_sb[hp : hp + 64, kblk, kb * 128 : (kb + 1) * 128],
                            qk_sb[hp : hp + 64, qblk, qb * TB : (qb + 1) * TB],
                            start=True,
                            stop=True,
                        )
                        att = work.tile([128, TB], bf16, tag="att", bufs=6)
                        # exp(score/sqrt(64)); softmax max-sub skipped (tiny scores)
                        nc.scalar.activation(att[:], ps_att[:], Exp, scale=0.125)
                        if kb >= qb * 4:
                            nc.vector.tensor_mul(
                                att[:], att[:], masks_sb[:, kb - qb * 4, :]
                            )
                        nc.tensor.matmul(
                            oe[:],
                            v_sb[:, kb, h, :],
                            att[:],
                            start=(kb == 0),
                            stop=(kb == nkb - 1),
                        )
                    # normalize: rows 0-63 are out^T, row 64 is the denominator
                    rden = work.tile([1, TB], f32, tag="rden", bufs=2)
                    nc.vector.reciprocal(rden[:], oe[64:65, :])
                    rdb = psum_bc.tile([64, TB], f32, tag="bc")
                    nc.tensor.matmul(rdb[:], ones_sb[:], rden[:], start=True, stop=True)
                    ou = work.tile([64, TB], bf16, tag="ou", bufs=2)
                    nc.scalar.copy(ou[:], oe[0:64, :])
                    nc.vector.tensor_mul(
                        out_sb[hp : hp + 64, qblk, qb * TB : (qb + 1) * TB],
                        ou[:],
                        rdb[:],
                    )

            # ---- phase 3: proj^T = Wp^T @ out^T + bp (partial) ----
            for ob in range(6):
                for tb in range(2):
                    ps_pr = psum_mm.tile([128, TB], f32, tag="mm")
                    for r in range(3):
                        nc.tensor.matmul(
                            ps_pr[:],
                            wp_sb[:, r, ob * 128 : (ob + 1) * 128],
                            out_sb[:, r, tb * TB : (tb + 1) * TB],
                            start=(r == 0),
                            stop=(r == 2),
                        )
                    res = work.tile([128, TB], f32, tag="res", bufs=3)
                    nc.vector.tensor_scalar_add(res[:], ps_pr[:], bp_sb[:, ob : ob + 1])
                    nc.sync.dma_start(
                        out[ob * 128 : (ob + 1) * 128, tb * TB : (tb + 1) * TB],
                        res[:],
                    )

    nc.compile()
    return nc


def _get_prog():
    global _prog
    if _prog is None:
        _prog = _build_program()
    return _prog


def make_in_maps(x, Wqkv, bqkv, Wproj, bproj):
    """Host-side sharding: per-core input dict."""
    x = np.asarray(x, dtype=np.float32)
    Wqkv = np.asarray(Wqkv, dtype=np.float32)
    bqkv = np.asarray(bqkv, dtype=np.float32)
    Wproj = np.asarray(Wproj, dtype=np.float32)
    bproj = np.asarray(bproj, dtype=np.float32)

    mask_list = []
    f = np.arange(TB)[None, :]
    p = np.arange(128)[:, None]
    for o in (0, 128, 256, 384):
        mask_list.append((f >= p + o))
    masks = np.concatenate(mask_list, axis=1).astype(BF16)  # [128, 2048]

    in_maps = []
    for c in range(NCORES):
        b, hg = c // 2, c % 2
        qcols = slice(hg * VC, (hg + 1) * VC)
        kcols = slice(C + hg * VC, C + (hg + 1) * VC)
        vcols = slice(2 * C + hg * VC, 2 * C + (hg + 1) * VC)
        wqk = np.concatenate([Wqkv[:, qcols], Wqkv[:, kcols]], axis=1)
        bqk = np.concatenate([bqkv[qcols], bqkv[kcols]])
        bp_c = bproj if hg == 0 else np.zeros_like(bproj)
        in_maps.append(
            {
                "xT": np.ascontiguousarray(x[b].T).astype(BF16),
                "wqk": np.ascontiguousarray(wqk).astype(BF16),
                "wv": np.ascontiguousarray(Wqkv[:, vcols]).astype(BF16),
                "wp": np.ascontiguousarray(Wproj[hg * VC : (hg + 1) * VC, :]).astype(
                    BF16
                ),
                "bqk": np.ascontiguousarray(bqk.reshape(6, 128).T).astype(np.float32),
                "bv": np.broadcast_to(bqkv[vcols], (128, VC)).copy().astype(np.float32),
                "bp": np.ascontiguousarray(bp_c.reshape(6, 128).T).astype(np.float32),
                "masks": masks,
            }
        )
    return in_maps


def gather_output(results):
    """results: per-core dict with 'out' [768, 1024] f32 partials."""
    outs = []
    for b in range(B):
        part = results[2 * b]["out"].astype(np.float32) + results[2 * b + 1][
            "out"
        ].astype(np.float32)
        outs.append(part.T)
    return np.stack(outs).astype(np.float32)


def run(inputs, trace=False):
    from concourse.bass_utils import run_bass_kernel_spmd

    nc = _get_prog()
    in_maps = make_in_maps(
        inputs["x"], inputs["Wqkv"], inputs["bqkv"], inputs["Wproj"], inputs["bproj"]
    )
    res = run_bass_kernel_spmd(nc, in_maps, list(range(NCORES)), trace=trace)
    return gather_output(res.results), res


def kernel(**inputs):
    out, _ = run(inputs, trace=False)
    return out


# revision 28
# speedup vs baseline: 1.7123x; 1.7123x over previous
"""Causal self-attention (B=4, T=1024, C=768, 12 heads) on 8 trn2 cores.

Sharding: core c = (batch b=c//2, head-group hg=c%2 of 6 heads).
Each core: QKV projection for its head-group (TP column split of Wqkv),
causal attention for 6 heads, partial output projection (TP row split of
Wproj). Host sums the two partials per batch (the all-reduce) and
transposes back.

Device-side layouts (contraction dim always on partitions, so no
on-device transposes are needed):
  x^T  [C=768, T=1024]    (prepared host-side)
  q^T/k^T = W^T x^T as [cols, T]  (lhsT=Wqk slice, rhs=x^T)
  v = x W_v as [T, cols]          (lhsT=x^T chunk, rhs=Wv)
  scoresT [T_k, T_q] = k_h q_h^T  (lhsT=k_h^T, rhs=q_h^T)
  softmax without max-subtraction (scores ~ N(0, 0.1); exp is safe),
  denominator via a ones-column appended to v (row 64 of att@[v|1]),
  out_h^T [64, T_q] = [v|1]^T attT (lhsT=v_ext chunk, rhs=attT chunk)
  proj^T [768, T] = Wp_hg^T out^T (lhsT=Wp slice, rhs=out^T)

Heads run in pairs at SBUF partition offsets 0/64 so the two K=64 QK
matmuls occupy distinct PE row-groups and run concurrently; their score
tiles share one 2-bank PSUM tile so exp is a single ACT op per block.
Staircase (diagonal) blocks are trimmed to the causally-live columns.
V is computed first and q^T/k^T in pair order so attention overlaps the
tail of the QKV projection. AV for block kb issues after QK for kb+1 so
the PE never waits on the exp chain. NOTE: custom DVE ops
(reciprocal_approx_fast) require base_partition 0 inputs on HW.
"""

import numpy as np
import ml_dtypes

B, T, C = 4, 1024, 768
NH, HD = 12, 64
HPC = NH // 2          # heads per core = 6
QKCOLS = 2 * HPC * HD  # 768 (q then k cols for this head group)
VC = HPC * HD          # 384
NCORES = 8
TB = 512               # matmul moving free-dim block
BF16 = ml_dtypes.bfloat16

_prog = None


def _build_program():
    import concourse.bass as bass
    import concourse.tile as tile
    from concourse import bacc, mybir

    f32 = mybir.dt.float32
    bf16 = mybir.dt.bfloat16

    nc = bacc.Bacc(
        "TRN2", target_bir_lowering=False, debug=False, enable_asserts=False
    )

    xT = nc.dram_tensor("xT", [C, T], bf16, kind="ExternalInput")
    wqk = nc.dram_tensor("wqk", [C, QKCOLS], bf16, kind="ExternalInput")
    wv = nc.dram_tensor("wv", [C, VC], bf16, kind="ExternalInput")
    wp = nc.dram_tensor("wp", [VC, C], bf16, kind="ExternalInput")
    fconsts = nc.dram_tensor("fconsts", [128, 6 + VC + 6], f32, kind="ExternalInput")
    hconsts = nc.dram_tensor("hconsts", [128, 4 * TB + 128], bf16, kind="ExternalInput")
    out = nc.dram_tensor("out", [C, T], bf16, kind="ExternalOutput")

    Exp = mybir.ActivationFunctionType.Exp

    with tile.TileContext(nc) as tc:
        with (
            tc.tile_pool(name="consts", bufs=1) as consts,
            tc.tile_pool(name="psum_ps", bufs=2, space="PSUM") as psum_ps,
            tc.tile_pool(name="psum_acc", bufs=2, space="PSUM") as psum_acc,
            tc.tile_pool(name="work", bufs=1) as work,
        ):
            # ---- load inputs into SBUF (x/w split per chunk for early overlap) ----
            xT_sb = consts.tile([128, 6, T], bf16)
            wv_sb = consts.tile([128, 6, VC], bf16)
            wqk_sb = consts.tile([128, 6, QKCOLS], bf16)
            xT_r = xT.rearrange("(a p) t -> p a t", p=128)
            wqk_r = wqk.rearrange("(a p) c -> p a c", p=128)
            wv_r = wv.rearrange("(a p) c -> p a c", p=128)
            wp_sb = consts.tile([128, 3, C], bf16)
            fc_sb = consts.tile([128, 6 + VC + 6], f32)
            bqk_sb = fc_sb[:, 0:6]
            bv_sb = fc_sb[:, 6 : 6 + VC]
            bp_sb = fc_sb[:, 6 + VC : 6 + VC + 6]
            hc_sb = consts.tile([128, 4 * TB + 128], bf16)
            masks_sb = hc_sb[:, 0 : 4 * TB].rearrange("p (m f) -> p m f", m=4)
            ident_sb = hc_sb[:, 4 * TB : 4 * TB + 128]
            # spread DMA issue (~600ns each) across the idle engines so the
            # first matmul operands land as early as possible
            eng = [nc.sync, nc.gpsimd, nc.scalar]
            for kc in range(6):
                eng[kc % 3].dma_start(xT_sb[:, kc, :], xT_r[:, kc, :])
                eng[kc % 3].dma_start(wv_sb[:, kc, :], wv_r[:, kc, :])
            for kc in range(6):
                eng[kc % 3].dma_start(wqk_sb[:, kc, :], wqk_r[:, kc, :])
            nc.gpsimd.dma_start(fc_sb[:], fconsts[:])
            nc.sync.dma_start(wp_sb[:], wp.rearrange("(a p) c -> p a c", p=128))
            nc.scalar.dma_start(hc_sb[:], hconsts[:])

            qk_sb = consts.tile([128, 6, T], bf16)   # q^T (blocks 0-2), k^T (3-5)
            v_sb = consts.tile([128, 8, HPC, HD + 1], bf16)  # [Tk chunk][head][v|1]
            out_sb = consts.tile([128, 3, T], bf16)  # attention out^T [384, T]

            nc.vector.memset(v_sb[:, :, :, HD : HD + 1], 1.0)

            # HAM warm-up: ~5us of dummy matmuls while the input DMAs land so
            # the PE clock-gate opens (1.2 -> 2.4 GHz) before real work starts
            wz = consts.tile([128, TB], bf16)
            nc.vector.memset(wz[:], 0.0)
            for w in range(24):
                ps_w = psum_ps.tile([128, 2, TB], f32, tag="ps", name="ps_w")
                nc.tensor.matmul(
                    ps_w[:, 0, :], wz[:, 0:128], wz[:], start=True, stop=True
                )

            # ---- phase 1a: v = x @ Wv + bv, in [T, cols] layout (first: frees
            # attention to start as soon as the pair's q^T/k^T land) ----
            for tk in range(8):
                ps_v = psum_ps.tile([128, 2, TB], f32, tag="ps", name="ps_v")
                for kc in range(6):
                    nc.tensor.matmul(
                        ps_v[:, 0, 0:VC],
                        xT_sb[:, kc, tk * 128 : (tk + 1) * 128],
                        wv_sb[:, kc, :],
                        start=(kc == 0),
                        stop=(kc == 5),
                    )
                nc.vector.tensor_add(
                    v_sb[:, tk, :, 0:HD],
                    ps_v[:, 0, 0:VC].rearrange("p (h d) -> p h d", h=HPC),
                    bv_sb.rearrange("p (h d) -> p h d", h=HPC),
                )

            # ---- phase 1b: q^T / k^T = Wqk^T @ x^T, [cols, T], pair order.
            # Both T-blocks per weight so the stationary operand is reused. ----
            for cb in (0, 3, 1, 4, 2, 5):
                ps_qk = psum_ps.tile([128, 2, TB], f32, tag="ps", name="ps_qk")
                for kc in range(6):
                    for tb in range(2):
                        nc.tensor.matmul(
                            ps_qk[:, tb, :],
                            wqk_sb[:, kc, cb * 128 : (cb + 1) * 128],
                            xT_sb[:, kc, tb * TB : (tb + 1) * TB],
                            start=(kc == 0),
                            stop=(kc == 5),
                        )
                nc.scalar.add(
                    qk_sb[:, cb, :].rearrange("p (a f) -> p a f", a=2),
                    ps_qk[:],
                    bqk_sb[:, cb : cb + 1],
                )

            # ---- phase 2 + 3: attention (qb outer), proj overlapped.
            # After the qb=0 half of all pairs finishes, the tb=0 half of the
            # projection runs while attention continues on qb=1.
            def proj_half(tb):
                for ob in range(6):
                    ps_pr = psum_ps.tile([128, 2, TB], f32, tag="ps", name="ps_pr")
                    for r in range(3):
                        nc.tensor.matmul(
                            ps_pr[:, 0, :],
                            wp_sb[:, r, ob * 128 : (ob + 1) * 128],
                            out_sb[:, r, tb * TB : (tb + 1) * TB],
                            start=(r == 0),
                            stop=(r == 2),
                        )
                    res = work.tile([128, TB], bf16, tag="res", bufs=3)
                    nc.scalar.add(res[:], ps_pr[:, 0, :], bp_sb[:, ob : ob + 1])
                    eng[ob % 3].dma_start(
                        out[ob * 128 : (ob + 1) * 128, tb * TB : (tb + 1) * TB],
                        res[:],
                    )

            pend = []  # (kb, o, att2, av_fn) blocks whose AV is not yet issued
            for j in range(3):
                for qb in range(2):
                    qblk, kblk = j, 3 + j
                    hA, hB = 2 * j, 2 * j + 1
                    nkb = 4 * (qb + 1)     # causal: T_k chunks needed
                    oe2 = psum_acc.tile([65, 2, TB], f32, tag="acc", name="oe2")

                    def qk_exp(kb, qblk=qblk, kblk=kblk, qb=qb):
                        stair = kb >= qb * 4
                        o = (kb - qb * 4) * 128 if stair else 0
                        qs = slice(qb * TB + o, (qb + 1) * TB)
                        ks = slice(kb * 128, (kb + 1) * 128)
                        ps2 = psum_ps.tile([128, 2, TB], f32, tag="ps", name="ps2")
                        nc.tensor.matmul(
                            ps2[:, 0, o:],
                            qk_sb[0:64, kblk, ks],
                            qk_sb[0:64, qblk, qs],
                            start=True,
                            stop=not stair,
                        )
                        nc.tensor.matmul(
                            ps2[:, 1, o:],
                            qk_sb[64:128, kblk, ks],
                            qk_sb[64:128, qblk, qs],
                            start=True,
                            stop=not stair,
                        )
                        if stair:
                            # accumulate the additive causal mask (0 / -3e4)
                            # via identity matmul; exp then underflows to 0
                            mi = kb - qb * 4
                            for i in range(2):
                                nc.tensor.matmul(
                                    ps2[:, i, o:],
                                    ident_sb[:],
                                    masks_sb[:, mi, o:],
                                    start=False,
                                    stop=True,
                                )
                        att2 = work.tile([128, 2, TB], bf16, tag="att", bufs=4)
                        # exp(score/8); softmax max-subtraction skipped (tiny scores)
                        nc.scalar.activation(
                            att2[:, :, o:], ps2[:, :, o:], Exp, scale=0.125
                        )
                        return o, att2

                    def av(kb, o, att2, oe2=oe2, hA=hA, hB=hB, nkb=nkb):
                        for i, h in ((0, hA), (1, hB)):
                            nc.tensor.matmul(
                                oe2[:, i, o:],
                                v_sb[:, kb, h, :],
                                att2[:, i, o:],
                                start=(kb == 0),
                                stop=(kb == nkb - 1),
                            )

                    # AV for a block issues only after the next QK (even
                    # across pair boundaries): the PE always has score-matmuls
                    # queued while ACT computes exp, so it never bubbles.
                    for kb in range(nkb):
                        item = (kb, *qk_exp(kb))
                        pend.append(lambda it=item, fn=av: fn(*it))
                        while len(pend) > 1:
                            pend.pop(0)()

                    def norm(oe2=oe2, qblk=qblk, qb=qb):
                        # normalize rows 0-63 by the denominator in row 64
                        den2 = work.tile([1, 2 * TB], f32, tag="den", bufs=2)
                        nc.vector.tensor_copy(
                            den2[:].rearrange("p (a f) -> p a f", a=2),
                            oe2[64:65, :, :],
                        )
                        rden2 = work.tile([1, 2 * TB], f32, tag="rden", bufs=2)
                        nc.vector.reciprocal_approx_fast(rden2[:], den2[:])
                        rdb2 = work.tile([64, 2 * TB], f32, tag="rdb", bufs=2)
                        nc.gpsimd.partition_broadcast(rdb2[:], rden2[:])
                        for i in range(2):
                            nc.vector.tensor_mul(
                                out_sb[
                                    i * 64 : (i + 1) * 64,
                                    qblk,
                                    qb * TB : (qb + 1) * TB,
                                ],
                                oe2[0:64, i, :],
                                rdb2[:, i * TB : (i + 1) * TB],
                            )

                    pend.append(norm)
            pend.pop(0)()   # last pair's deferred AV
            proj_half(0)    # tb=0 projection only needs qb=0 halves: overlaps
            pend.pop(0)()   # the last pair's normalization chain
            proj_half(1)

    nc.compile()
    return nc


def _get_prog():
    global _prog
    if _prog is None:
        _prog = _build_program()
    return _prog


def make_in_maps(x, Wqkv, bqkv, Wproj, bproj):
    """Host-side sharding: per-core input dict."""
    x = np.asarray(x, dtype=np.float32)
    Wqkv = np.asarray(Wqkv, dtype=np.float32)
    bqkv = np.asarray(bqkv, dtype=np.float32)
    Wproj = np.asarray(Wproj, dtype=np.float32)
    bproj = np.asarray(bproj, dtype=np.float32)

    f = np.arange(TB)[None, :]
    p = np.arange(128)[:, None]
    masks = np.concatenate(
        [np.where(f >= p + o, 0.0, -30000.0) for o in (0, 128, 256, 384)], axis=1
    ).astype(np.float32)  # [128, 2048] additive causal masks
    ident = np.eye(128, dtype=np.float32)
    hconsts = np.concatenate([masks, ident], axis=1).astype(BF16)

    in_maps = []
    for c in range(NCORES):
        b, hg = c // 2, c % 2
        qcols = slice(hg * VC, (hg + 1) * VC)
        kcols = slice(C + hg * VC, C + (hg + 1) * VC)
        vcols = slice(2 * C + hg * VC, 2 * C + (hg + 1) * VC)
        wqk_c = np.concatenate([Wqkv[:, qcols], Wqkv[:, kcols]], axis=1)
        bqk_c = np.concatenate([bqkv[qcols], bqkv[kcols]])
        bp_c = bproj if hg == 0 else np.zeros_like(bproj)
        in_maps.append(
            {
                "xT": np.ascontiguousarray(x[b].T).astype(BF16),
                "wqk": np.ascontiguousarray(wqk_c).astype(BF16),
                "wv": np.ascontiguousarray(Wqkv[:, vcols]).astype(BF16),
                "wp": np.ascontiguousarray(Wproj[hg * VC : (hg + 1) * VC, :]).astype(
                    BF16
                ),
                "fconsts": np.concatenate(
                    [
                        bqk_c.reshape(6, 128).T,
                        np.broadcast_to(bqkv[vcols], (128, VC)),
                        bp_c.reshape(6, 128).T,
                    ],
                    axis=1,
                ).astype(np.float32),
                "hconsts": hconsts,
            }
        )
    return in_maps


def gather_output(results):
    """results: per-core dict with 'out' [768, 1024] partials."""
    outs = []
    for b in range(B):
        part = results[2 * b]["out"].astype(np.float32) + results[2 * b + 1][
            "out"
        ].astype(np.float32)
        outs.append(part.T)
    return np.stack(outs).astype(np.float32)


def run(inputs, trace=False):
    from concourse.bass_utils import run_bass_kernel_spmd

    nc = _get_prog()
    in_maps = make_in_maps(
        inputs["x"], inputs["Wqkv"], inputs["bqkv"], inputs["Wproj"], inputs["bproj"]
    )
    res = run_bass_kernel_spmd(nc, in_maps, list(range(NCORES)), trace=trace)
    return gather_output(res.results), res


def kernel(**inputs):
    out, _ = run(inputs, trace=False)
    return out


# revision 33
# speedup vs baseline: 1.8645x; 1.0889x over previous
"""Causal self-attention (B=4, T=1024, C=768, 12 heads) on 8 trn2 cores.

Sharding: core c = (batch b=c//2, head-group hg=c%2 of 6 heads).
Each core: QKV projection for its head-group (TP column split of Wqkv),
causal attention for 6 heads, partial output projection (TP row split of
Wproj). Host sums the two partials per batch (the all-reduce) and
transposes back.

Device-side layouts (contraction dim always on partitions, so no
on-device transposes are needed):
  x^T  [C=768, T=1024]    (prepared host-side)
  q^T/k^T = W^T x^T as [cols, T]  (lhsT=Wqk slice, rhs=x^T)
  v = x W_v as [T, cols]          (lhsT=x^T chunk, rhs=Wv)
  scoresT [T_k, T_q] = k_h q_h^T  (lhsT=k_h^T, rhs=q_h^T)
  softmax without max-subtraction (scores ~ N(0, 0.1); exp is safe),
  denominator via a ones-column appended to v (row 64 of att@[v|1]),
  out_h^T [64, T_q] = [v|1]^T attT (lhsT=v_ext chunk, rhs=attT chunk)
  proj^T [768, T] = Wp_hg^T out^T (lhsT=Wp slice, rhs=out^T)

Heads run in pairs at SBUF partition offsets 0/64 so the two K=64 QK
matmuls occupy distinct PE row-groups and run concurrently; their score
tiles share one 2-bank PSUM tile so exp is a single ACT op per block.
Staircase (diagonal) blocks are trimmed to the causally-live columns.
V is computed first and q^T/k^T in pair order so attention overlaps the
tail of the QKV projection. AV for block kb issues after QK for kb+1 so
the PE never waits on the exp chain. NOTE: custom DVE ops
(reciprocal_approx_fast) require base_partition 0 inputs on HW.
"""

import numpy as np
import ml_dtypes

B, T, C = 4, 1024, 768
NH, HD = 12, 64
HPC = NH // 2          # heads per core = 6
QKCOLS = 2 * HPC * HD  # 768 (q then k cols for this head group)
VC = HPC * HD          # 384
NCORES = 8
TB = 512               # matmul moving free-dim block
BF16 = ml_dtypes.bfloat16

_prog = None


def _build_program():
    import concourse.bass as bass
    import concourse.tile as tile
    from concourse import bacc, mybir

    f32 = mybir.dt.float32
    bf16 = mybir.dt.bfloat16

    nc = bacc.Bacc(
        "TRN2", target_bir_lowering=False, debug=False, enable_asserts=False
    )

    xT = nc.dram_tensor("xT", [C, T], bf16, kind="ExternalInput")
    wqk = nc.dram_tensor("wqk", [C, QKCOLS], bf16, kind="ExternalInput")
    wv = nc.dram_tensor("wv", [C, VC], bf16, kind="ExternalInput")
    wp = nc.dram_tensor("wp", [VC, C], bf16, kind="ExternalInput")
    fconsts = nc.dram_tensor("fconsts", [128, 6 + VC + 6], f32, kind="ExternalInput")
    hconsts = nc.dram_tensor("hconsts", [128, 4 * TB + 128], bf16, kind="ExternalInput")
    out = nc.dram_tensor("out", [C, T], bf16, kind="ExternalOutput")

    Exp = mybir.ActivationFunctionType.Exp

    with tile.TileContext(nc) as tc:
        with (
            tc.tile_pool(name="consts", bufs=1) as consts,
            tc.tile_pool(name="psum_ps", bufs=3, space="PSUM") as psum_ps,
            tc.tile_pool(name="psum_acc", bufs=1, space="PSUM") as psum_acc,
            tc.tile_pool(name="work", bufs=1) as work,
        ):
            # ---- load inputs into SBUF (x/w split per chunk for early overlap) ----
            xT_sb = consts.tile([128, 6, T], bf16)
            wv_sb = consts.tile([128, 6, VC], bf16)
            wqk_sb = consts.tile([128, 6, QKCOLS], bf16)
            xT_r = xT.rearrange("(a p) t -> p a t", p=128)
            wqk_r = wqk.rearrange("(a p) c -> p a c", p=128)
            wv_r = wv.rearrange("(a p) c -> p a c", p=128)
            wp_sb = consts.tile([128, 3, C], bf16)
            fc_sb = consts.tile([128, 6 + VC + 6], f32)
            bqk_sb = fc_sb[:, 0:6]
            bv_sb = fc_sb[:, 6 : 6 + VC]
            bp_sb = fc_sb[:, 6 + VC : 6 + VC + 6]
            hc_sb = consts.tile([128, 4 * TB + 128], bf16)
            masks_sb = hc_sb[:, 0 : 4 * TB].rearrange("p (m f) -> p m f", m=4)
            ident_sb = hc_sb[:, 4 * TB : 4 * TB + 128]
            # spread DMA issue (~600ns each) across the idle engines so the
            # first matmul operands land as early as possible
            eng = [nc.sync, nc.gpsimd, nc.scalar]
            for kc in range(6):
                eng[kc % 3].dma_start(xT_sb[:, kc, :], xT_r[:, kc, :])
                eng[kc % 3].dma_start(wv_sb[:, kc, :], wv_r[:, kc, :])
            for kc in range(6):
                eng[kc % 3].dma_start(wqk_sb[:, kc, :], wqk_r[:, kc, :])
            nc.gpsimd.dma_start(fc_sb[:], fconsts[:])
            nc.sync.dma_start(wp_sb[:], wp.rearrange("(a p) c -> p a c", p=128))
            nc.scalar.dma_start(hc_sb[:], hconsts[:])

            qk_sb = consts.tile([128, 6, T], bf16)   # q^T (blocks 0-2), k^T (3-5)
            v_sb = consts.tile([128, 8, HPC, HD + 1], bf16)  # [Tk chunk][head][v|1]
            out_sb = consts.tile([128, 3, T], bf16)  # attention out^T [384, T]

            nc.vector.memset(v_sb[:, :, :, HD : HD + 1], 1.0)

            # HAM warm-up: ~5us of dummy matmuls while the input DMAs land so
            # the PE clock-gate opens (1.2 -> 2.4 GHz) before real work starts
            wz = consts.tile([128, TB], bf16)
            nc.vector.memset(wz[:], 0.0)
            for w in range(24):
                ps_w = psum_ps.tile([128, 2, TB], f32, tag="ps", name="ps_w")
                nc.tensor.matmul(
                    ps_w[:, 0, :], wz[:, 0:128], wz[:], start=True, stop=True
                )

            # ---- phase 1a: v = x @ Wv + bv, in [T, cols] layout (first: frees
            # attention to start as soon as the pair's q^T/k^T land) ----
            for tk in range(8):
                ps_v = psum_ps.tile([128, 2, TB], f32, tag="ps", name="ps_v")
                for kc in range(6):
                    nc.tensor.matmul(
                        ps_v[:, 0, 0:VC],
                        xT_sb[:, kc, tk * 128 : (tk + 1) * 128],
                        wv_sb[:, kc, :],
                        start=(kc == 0),
                        stop=(kc == 5),
                    )
                nc.vector.tensor_add(
                    v_sb[:, tk, :, 0:HD],
                    ps_v[:, 0, 0:VC].rearrange("p (h d) -> p h d", h=HPC),
                    bv_sb.rearrange("p (h d) -> p h d", h=HPC),
                )

            # ---- phase 1b: q^T / k^T = Wqk^T @ x^T, [cols, T], pair order.
            # Both T-blocks per weight so the stationary operand is reused. ----
            for cb in (0, 3, 1, 4, 2, 5):
                ps_qk = psum_ps.tile([128, 2, TB], f32, tag="ps", name="ps_qk")
                for kc in range(6):
                    for tb in range(2):
                        nc.tensor.matmul(
                            ps_qk[:, tb, :],
                            wqk_sb[:, kc, cb * 128 : (cb + 1) * 128],
                            xT_sb[:, kc, tb * TB : (tb + 1) * TB],
                            start=(kc == 0),
                            stop=(kc == 5),
                        )
                nc.scalar.add(
                    qk_sb[:, cb, :].rearrange("p (a f) -> p a f", a=2),
                    ps_qk[:],
                    bqk_sb[:, cb : cb + 1],
                )

            # ---- phase 2 + 3: attention (qb outer), proj overlapped.
            # After the qb=0 half of all pairs finishes, the tb=0 half of the
            # projection runs while attention continues on qb=1.
            def proj_half(tb):
                for ob in range(6):
                    ps_pr = psum_ps.tile([128, 2, TB], f32, tag="ps", name="ps_pr")
                    for r in range(3):
                        nc.tensor.matmul(
                            ps_pr[:, 0, :],
                            wp_sb[:, r, ob * 128 : (ob + 1) * 128],
                            out_sb[:, r, tb * TB : (tb + 1) * TB],
                            start=(r == 0),
                            stop=(r == 2),
                        )
                    res = work.tile([128, TB], bf16, tag="res", bufs=3)
                    nc.scalar.add(res[:], ps_pr[:, 0, :], bp_sb[:, ob : ob + 1])
                    eng[ob % 3].dma_start(
                        out[ob * 128 : (ob + 1) * 128, tb * TB : (tb + 1) * TB],
                        res[:],
                    )

            pend = []  # deferred AV / normalization tasks
            for j in range(3):
                for qb in range(2):
                    qblk, kblk = j, 3 + j
                    hA, hB = 2 * j, 2 * j + 1
                    nkb = 4 * (qb + 1)     # causal: T_k chunks needed
                    oe2 = psum_acc.tile([65, 2, TB], f32, tag="acc", name="oe2")

                    def qk_exp(kb, qblk=qblk, kblk=kblk, qb=qb):
                        stair = kb >= qb * 4
                        o = (kb - qb * 4) * 128 if stair else 0
                        qs = slice(qb * TB + o, (qb + 1) * TB)
                        ks = slice(kb * 128, (kb + 1) * 128)
                        ps2 = psum_ps.tile([128, 2, TB], f32, tag="ps", name="ps2")
                        nc.tensor.matmul(
                            ps2[:, 0, o:],
                            qk_sb[0:64, kblk, ks],
                            qk_sb[0:64, qblk, qs],
                            start=True,
                            stop=not stair,
                        )
                        nc.tensor.matmul(
                            ps2[:, 1, o:],
                            qk_sb[64:128, kblk, ks],
                            qk_sb[64:128, qblk, qs],
                            start=True,
                            stop=not stair,
                        )
                        if stair:
                            # accumulate the additive causal mask (0 / -3e4)
                            # via identity matmul; exp then underflows to 0
                            mi = kb - qb * 4
                            for i in range(2):
                                nc.tensor.matmul(
                                    ps2[:, i, o:],
                                    ident_sb[:],
                                    masks_sb[:, mi, o:],
                                    start=False,
                                    stop=True,
                                )
                        att2 = work.tile([128, 2, TB], bf16, tag="att", bufs=6)
                        # exp(score/8); softmax max-subtraction skipped (tiny scores)
                        nc.scalar.activation(
                            att2[:, :, o:], ps2[:, :, o:], Exp, scale=0.125
                        )
                        return o, att2

                    def av(kb, o, att2, oe2=oe2, hA=hA, hB=hB, nkb=nkb):
                        for i, h in ((0, hA), (1, hB)):
                            nc.tensor.matmul(
                                oe2[:, i, o:],
                                v_sb[:, kb, h, :],
                                att2[:, i, o:],
                                start=(kb == 0),
                                stop=(kb == nkb - 1),
                            )

                    # AV for a block issues only after the next QK (even
                    # across pair boundaries): the PE always has score-matmuls
                    # queued while ACT computes exp, so it never bubbles.
                    for kb in range(nkb):
                        item = (kb, *qk_exp(kb))
                        pend.append(lambda it=item, fn=av: fn(*it))
                        while len(pend) > 2:
                            pend.pop(0)()

                    def norm(oe2=oe2, qblk=qblk, qb=qb):
                        # single drain copy frees the one PSUM acc slot ASAP;
                        # the rest of the chain runs from SBUF (den bounced to
                        # partition 0 for the custom reciprocal; multiplies on
                        # the otherwise-idle GpSimd)
                        den2 = work.tile([1, 2 * TB], f32, tag="den", bufs=2)
                        nc.vector.tensor_copy(
                            den2[:].rearrange("p (a f) -> p a f", a=2),
                            oe2[64:65, :, :],
                        )
                        oecp = work.tile([64, 2, TB], f32, tag="oecp", bufs=2)
                        nc.vector.tensor_copy(oecp[:], oe2[0:64, :, :])
                        rden2 = work.tile([1, 2 * TB], f32, tag="rden", bufs=2)
                        nc.vector.reciprocal_approx_fast(rden2[:], den2[:])
                        rdb2 = work.tile([64, 2 * TB], f32, tag="rdb", bufs=2)
                        nc.gpsimd.partition_broadcast(rdb2[:], rden2[:])
                        for i in range(2):
                            nc.vector.tensor_mul(
                                out_sb[
                                    i * 64 : (i + 1) * 64,
                                    qblk,
                                    qb * TB : (qb + 1) * TB,
                                ],
                                oecp[:, i, :],
                                rdb2[:, i * TB : (i + 1) * TB],
                            )

                    pend.append(norm)
            while len(pend) > 1:
                pend.pop(0)()   # remaining deferred AVs
            proj_half(0)    # tb=0 projection only needs qb=0 halves: overlaps
            pend.pop(0)()   # the last pair's normalization chain
            proj_half(1)

    nc.compile()
    return nc


def _get_prog():
    global _prog
    if _prog is None:
        _prog = _build_program()
    return _prog


def make_in_maps(x, Wqkv, bqkv, Wproj, bproj):
    """Host-side sharding: per-core input dict."""
    x = np.asarray(x, dtype=np.float32)
    Wqkv = np.asarray(Wqkv, dtype=np.float32)
    bqkv = np.asarray(bqkv, dtype=np.float32)
    Wproj = np.asarray(Wproj, dtype=np.float32)
    bproj = np.asarray(bproj, dtype=np.float32)

    f = np.arange(TB)[None, :]
    p = np.arange(128)[:, None]
    masks = np.concatenate(
        [np.where(f >= p + o, 0.0, -30000.0) for o in (0, 128, 256, 384)], axis=1
    ).astype(np.float32)  # [128, 2048] additive causal masks
    ident = np.eye(128, dtype=np.float32)
    hconsts = np.concatenate([masks, ident], axis=1).astype(BF16)

    in_maps = []
    for c in range(NCORES):
        b, hg = c // 2, c % 2
        qcols = slice(hg * VC, (hg + 1) * VC)
        kcols = slice(C + hg * VC, C + (hg + 1) * VC)
        vcols = slice(2 * C + hg * VC, 2 * C + (hg + 1) * VC)
        wqk_c = np.concatenate([Wqkv[:, qcols], Wqkv[:, kcols]], axis=1)
        bqk_c = np.concatenate([bqkv[qcols], bqkv[kcols]])
        bp_c = bproj if hg == 0 else np.zeros_like(bproj)
        in_maps.append(
            {
                "xT": np.ascontiguousarray(x[b].T).astype(BF16),
                "wqk": np.ascontiguousarray(wqk_c).astype(BF16),
                "wv": np.ascontiguousarray(Wqkv[:, vcols]).astype(BF16),
                "wp": np.ascontiguousarray(Wproj[hg * VC : (hg + 1) * VC, :]).astype(
                    BF16
                ),
                "fconsts": np.concatenate(
                    [
                        bqk_c.reshape(6, 128).T,
                        np.broadcast_to(bqkv[vcols], (128, VC)),
                        bp_c.reshape(6, 128).T,
                    ],
                    axis=1,
                ).astype(np.float32),
                "hconsts": hconsts,
            }
        )
    return in_maps


def gather_output(results):
    """results: per-core dict with 'out' [768, 1024] partials."""
    outs = []
    for b in range(B):
        part = results[2 * b]["out"].astype(np.float32) + results[2 * b + 1][
            "out"
        ].astype(np.float32)
        outs.append(part.T)
    return np.stack(outs).astype(np.float32)


def run(inputs, trace=False):
    from concourse.bass_utils import run_bass_kernel_spmd

    nc = _get_prog()
    in_maps = make_in_maps(
        inputs["x"], inputs["Wqkv"], inputs["bqkv"], inputs["Wproj"], inputs["bproj"]
    )
    res = run_bass_kernel_spmd(nc, in_maps, list(range(NCORES)), trace=trace)
    return gather_output(res.results), res


def kernel(**inputs):
    out, _ = run(inputs, trace=False)
    return out
